# revision 1
# baseline (speedup 1.0000x reference)
"""Trainium2 Bass kernel for the LogRatio loss (nn_LogRatio_14104672600201).

Math: the reference loss factorizes. Every masked reduction over
logsim[j, l] = log((X @ X.T)[j, l] + eps) has a mask that depends on j only
through targets[j] in [0, 64). So each row-reduction becomes a small GEMM
logsim @ Q with Q[l, g] built on host from labels, followed by a per-row
one-hot select at g = targets[j]:

  Q columns: [ P (one-hot of t_l) | W0T | W1T ]  (192 cols, zero-padded to 256)
  X1[j, g] = sum_l logsim[j, l]   * Q[l, g]
  X2[j, g] = sum_l logsim[j, l]^2 * Q[l, g]

  S1 = X1[j, t_j (P)] - diag_j          S2 = X2[j, t_j (P)] - diag_j^2
  A0 = H0[t_j]                          A1 = X1[j, t_j (W0)] + 0.1 * H1[t_j]
  A2 = X2[j, t_j (W0)] + 0.2 * X1[j, t_j (W1)] + 0.01 * H2[t_j]
  c  = cnt[t_j] - 1                     diag_j = log(||x_j||^2 + eps)
  loss = sum_j  S2 * A0 - 2 * S1 * A1 + c * A2

Sharding: data-parallel over j across 8 cores (512 rows each). Every core
holds the full xT (needed for the l dimension anyway) and computes its slab
of sim as [l_tile(128) x j_half(256)] PSUM tiles, so the reduction matmuls
get logsim already l-major (no on-chip transpose). Per-j partial losses are
DMA'd out and summed on host (the "all-reduce" of a scalar).

All matmuls use float32r (1 cycle/row at N >= 256, ~12-bit mantissa). To
keep that rounding harmless, the kernel works on the SHIFTED log
y = logsim - s (s ~ 3.5, folded into the Ln as Ln(sim * e^-s), so y is in
[-0.25, 0.45] and rounds ~10x finer). The shift terms are reconstructed
exactly in the epilogue from host-side tables:
  sum w*ls   = sum w*y  + s*sum(w)
  sum w*ls^2 = sum w*y^2 + 2s*sum(w*y) + s^2*sum(w)
"""

import numpy as np

N, D, KK, C = 4096, 128, 4, 64
NCORES = 8
JSH = N // NCORES          # 512 j rows per core
JH = 2                     # j-halves per core (256 cols each)
JCH = 4                    # j-chunks of 128 per core
LT = N // 128              # 32 l-tiles
QW = 256                   # padded Q width
NTAB = 6                   # tab columns in aux
EPS = 1e-6
OMEGA = 0.1
KSC = float(np.float32(np.exp(-3.5)))        # Ln input scale (exactly f32)
SHIFT = float(-np.log(np.float64(KSC)))      # effective shift s = -ln(KSC)

_CACHE = {}


def _build_nc():
    import bass_rust
    import concourse.bass as bass
    import concourse.bacc as bacc
    import concourse.mybir as mybir
    import concourse.tile as tile
    from contextlib import ExitStack

    f32 = mybir.dt.float32
    f32r = mybir.dt.float32r
    Ln = mybir.ActivationFunctionType.Ln
    mult = mybir.AluOpType.mult
    add = mybir.AluOpType.add
    AxX = mybir.AxisListType.X

    nc = bacc.Bacc("TRN2", target_bir_lowering=False, debug=False)
    xt = nc.dram_tensor("xt", [D, N], f32r, kind="ExternalInput")
    q = nc.dram_tensor("q", [LT, 128, QW], f32r, kind="ExternalInput")
    # aux columns: [0:64] one-hot of t_j, [64:70] tables, [70:198] x rows
    aux = nc.dram_tensor("aux", [JSH, C + NTAB + D], f32, kind="ExternalInput")
    lout = nc.dram_tensor("lout", [128, JCH], f32, kind="ExternalOutput")

    with tile.TileContext(nc) as tc, ExitStack() as ctx:
        cpool = ctx.enter_context(tc.tile_pool(name="const", bufs=1))
        lsp = ctx.enter_context(tc.tile_pool(name="lsp", bufs=1))
        work = ctx.enter_context(tc.tile_pool(name="work", bufs=4))
        small = ctx.enter_context(tc.tile_pool(name="small", bufs=2))
        psim = ctx.enter_context(tc.tile_pool(name="psim", bufs=3, space="PSUM"))
        px = ctx.enter_context(tc.tile_pool(name="px", bufs=1, space="PSUM"))

        # ---- constants: xT and Q resident in SBUF ----
        xt_sb = cpool.tile([D, N], f32r, tag="xt")
        for cchunk in range(4):
            sl = bass.ts(cchunk, 1024)
            nc.sync.dma_start(xt_sb[:, sl], xt[:, sl])
        q_sb = []
        q_dma = []
        for lt in range(LT):
            qt = cpool.tile([128, QW], f32r, tag=f"q{lt}", name=f"q_sb{lt}")
            q_dma.append(nc.sync.dma_start(qt[:], q[lt]))
            q_sb.append(qt)

        lbuf = cpool.tile([128, JCH], f32, tag="lbuf")

        for jh in range(JH):
            x1p = [
                px.tile([128, QW], f32, tag=f"x1_{i}", name=f"x1_{jh}_{i}")
                for i in range(2)
            ]
            x2p = [
                px.tile([128, QW], f32, tag=f"x2_{i}", name=f"x2_{jh}_{i}")
                for i in range(2)
            ]
            rhs_j = xt_sb[:, bass.ts(jh, 256)]
            for lt in range(LT):
                simp = psim.tile([128, 256], f32)
                nc.tensor.matmul(
                    simp[:],
                    xt_sb[:, bass.ts(lt, 128)],
                    rhs_j,
                    start=True,
                    stop=True,
                )
                ls = work.tile([128, 256], f32r, tag="ls")
                nc.scalar.activation(ls[:], simp[:], Ln, scale=KSC)
                ls2 = work.tile([128, 256], f32r, tag="ls2")
                nc.vector.tensor_mul(ls2[:], ls[:], ls[:])
                qr_ = q_sb[lt][:]
                for jc01 in range(2):
                    sl = bass.ts(jc01, 128)
                    nc.tensor.matmul(
                        x2p[jc01][:], ls2[:, sl], qr_,
                        start=(lt == 0), stop=(lt == LT - 1),
                    )
                    nc.tensor.matmul(
                        x1p[jc01][:], ls[:, sl], qr_,
                        start=(lt == 0), stop=(lt == LT - 1),
                    )

            # ---- epilogue per 128-row j-chunk ----
            for jc01 in range(2):
                jc = jh * 2 + jc01
                jsl = bass.ts(jc, 128)
                aux_t = small.tile(
                    [128, C + NTAB + D], f32, tag=f"aux{jc}", name=f"aux{jc}"
                )
                nc.sync.dma_start(aux_t[:], aux[jsl, :])
                pj_t = aux_t[:, 0:C]
                tab_t = aux_t[:, C : C + NTAB]
                xj_t = aux_t[:, C + NTAB : C + NTAB + D]

                sels = []
                for name, src in (
                    ("s1p", x1p[jc01][:, 0:64]),
                    ("s1w0", x1p[jc01][:, 64:128]),
                    ("s1w1", x1p[jc01][:, 128:192]),
                    ("s2p", x2p[jc01][:, 0:64]),
                    ("s2w0", x2p[jc01][:, 64:128]),
                ):
                    scr = small.tile(
                        [128, C], f32, tag=f"scr{jc}_{name}", name=f"scr_{jc}_{name}"
                    )
                    sel = small.tile(
                        [128, 1], f32, tag=f"{name}{jc}", name=f"{name}_{jc}"
                    )
                    nc.vector.tensor_mul(scr[:], src, pj_t)
                    nc.vector.reduce_sum(sel[:], scr[:], axis=AxX)
                    sels.append(sel)
                s1p, s1w0, s1w1, s2p, s2w0 = sels

                scr2 = small.tile([128, D], f32, tag=f"scr2{jc}", name=f"scr2{jc}")
                nrm = small.tile([128, 1], f32, tag=f"nrm{jc}", name=f"nrm{jc}")
                nc.vector.tensor_mul(scr2[:], xj_t, xj_t)
                nc.vector.reduce_sum(nrm[:], scr2[:], axis=AxX)
                diag = small.tile([128, 1], f32, tag=f"diag{jc}", name=f"diag{jc}")
                nc.scalar.activation(diag[:], nrm[:], Ln)
                diag2 = small.tile([128, 1], f32, tag=f"diag2{jc}", name=f"diag2{jc}")
                nc.vector.tensor_mul(diag2[:], diag[:], diag[:])

                # shift reconstruction: sels are y / y^2 sums, y = ls - s
                # S1 = yP + s*cnt - diag
                s1 = small.tile([128, 1], f32, tag=f"s1{jc}", name=f"s1{jc}")
                nc.vector.tensor_add(s1[:], s1p[:], tab_t[:, 2:3])
                nc.vector.tensor_sub(s1[:], s1[:], diag[:])
                # S2 = y2P + 2s*yP + s^2*cnt - diag^2
                s2 = small.tile([128, 1], f32, tag=f"s2{jc}", name=f"s2{jc}")
                nc.vector.scalar_tensor_tensor(
                    out=s2[:], in0=s1p[:], scalar=2.0 * SHIFT, in1=s2p[:],
                    op0=mult, op1=add,
                )
                nc.vector.tensor_add(s2[:], s2[:], tab_t[:, 3:4])
                nc.vector.tensor_sub(s2[:], s2[:], diag2[:])
                # A1 = yW0 + (s*H0 + 0.1*H1)
                a1 = small.tile([128, 1], f32, tag=f"a1{jc}", name=f"a1{jc}")
                nc.vector.tensor_add(a1[:], s1w0[:], tab_t[:, 4:5])
                # A2 = y2W0 + 2s*yW0 + 0.2*yW1 + (s^2*H0 + 0.2s*H1 + 0.01*H2)
                a2 = small.tile([128, 1], f32, tag=f"a2{jc}", name=f"a2{jc}")
                nc.vector.scalar_tensor_tensor(
                    out=a2[:], in0=s1w0[:], scalar=2.0 * SHIFT, in1=s2w0[:],
                    op0=mult, op1=add,
                )
                nc.vector.scalar_tensor_tensor(
                    out=a2[:], in0=s1w1[:], scalar=0.2, in1=a2[:],
                    op0=mult, op1=add,
                )
                nc.vector.tensor_add(a2[:], a2[:], tab_t[:, 5:6])
                # L = s2 * A0 + c * a2 - 2 * s1 * a1
                u = small.tile([128, 1], f32, tag=f"u{jc}", name=f"u{jc}")
                nc.vector.tensor_mul(u[:], s2[:], tab_t[:, 1:2])
                w = small.tile([128, 1], f32, tag=f"w{jc}", name=f"w{jc}")
                nc.vector.tensor_mul(w[:], a2[:], tab_t[:, 0:1])
                v = small.tile([128, 1], f32, tag=f"v{jc}", name=f"v{jc}")
                nc.vector.tensor_mul(v[:], s1[:], a1[:])
                nc.vector.tensor_add(u[:], u[:], w[:])
                nc.vector.scalar_tensor_tensor(
                    out=lbuf[:, jc : jc + 1], in0=v[:], scalar=-2.0,
                    in1=u[:], op0=mult, op1=add,
                )

        nc.sync.dma_start(lout[:], lbuf[:])
    nc.compile()
    return nc


def _host_prep(inputs, labels):
    x = np.ascontiguousarray(np.asarray(inputs, dtype=np.float32))
    lab = np.asarray(labels)
    t = lab[:, 0]

    m = np.arange(KK)
    om = np.float32(OMEGA)
    lp = (
        np.log(np.float32(OMEGA + EPS))
        - np.log((om ** (KK - m + 1)).astype(np.float32) + np.float32(EPS))
    ).astype(np.float32)

    gr = np.arange(C)
    eq = lab[None, :, :] == gr[:, None, None]          # [C, N, KK]
    nm = np.stack(
        [
            ~eq[:, :, 3],
            eq[:, :, 3] & ~eq[:, :, 2],
            eq[:, :, 2] & ~eq[:, :, 1],
            eq[:, :, 1] & ~eq[:, :, 0],
        ]
    ).astype(np.float32)                                # [KK, C, N]
    w0 = nm.sum(0)
    w1 = np.einsum("m,mcl->cl", lp, nm).astype(np.float32)
    w2 = np.einsum("m,mcl->cl", lp * lp, nm).astype(np.float32)
    ph = (t[:, None] == gr[None, :]).astype(np.float32)  # [N, C] one-hot t_l

    qm = np.zeros((N, QW), dtype=np.float32)
    qm[:, 0:C] = ph
    qm[:, C : 2 * C] = w0.T
    qm[:, 2 * C : 3 * C] = w1.T

    h0 = w0.sum(1)
    h1 = w1.sum(1)
    h2 = w2.sum(1)
    cnt = ph.sum(0)
    s = np.float64(SHIFT)
    tab = np.stack(
        [
            cnt[t] - 1.0,
            h0[t],
            s * cnt[t],
            s * s * cnt[t],
            s * h0[t] + 0.1 * h1[t],
            s * s * h0[t] + 0.2 * s * h1[t] + 0.01 * h2[t],
        ],
        axis=1,
    ).astype(np.float32)                                # [N, NTAB]

    xt = np.ascontiguousarray(x.T)                       # [D, N]
    auxf = np.concatenate([ph, tab, x], axis=1).astype(np.float32)  # [N, 198]
    in_maps = []
    for cid in range(NCORES):
        sl = slice(cid * JSH, (cid + 1) * JSH)
        # rotate the l axis so this core's own j-shard sits at columns
        # 0:JSH — the kernel always matmuls against xt[:, 0:512]; the l
        # reduction (over all 4096) is rotation-invariant as long as q's
        # rows rotate identically.
        xtc = np.ascontiguousarray(np.roll(xt, -cid * JSH, axis=1))
        qc = np.ascontiguousarray(
            np.roll(qm, -cid * JSH, axis=0).reshape(LT, 128, QW)
        )
        in_maps.append(
            {
                "xt": xtc,
                "q": qc,
                "aux": np.ascontiguousarray(auxf[sl]),
            }
        )
    return in_maps


def _run(inputs, labels, trace=False, tmpdir=None):
    from concourse.bass_utils import run_bass_kernel_spmd

    if "nc" not in _CACHE:
        _CACHE["nc"] = _build_nc()
    in_maps = _host_prep(inputs, labels)
    res = run_bass_kernel_spmd(
        _CACHE["nc"], in_maps, core_ids=list(range(NCORES)),
        trace=trace, tmpdir=tmpdir,
    )
    loss = np.float64(0.0)
    for r in res.results:
        loss += r["lout"].astype(np.float64).sum()
    return np.array(loss, dtype=np.float32), res


def kernel(inputs, labels):
    out, _ = _run(inputs, labels, trace=False)
    return out



# revision 5
# speedup vs baseline: 1.2228x; 1.2228x over previous
"""Trainium2 Bass kernel for the LogRatio loss (nn_LogRatio_14104672600201).

Math: the reference loss factorizes. Every masked reduction over
logsim[j, l] = log((X @ X.T)[j, l] + eps) has a mask that depends on j only
through targets[j] in [0, 64), so each row-reduction becomes a small GEMM
against matrices built on host from labels. After folding all shift / diag /
count terms into host-precomputed coefficients, the per-row loss is

  loss_j = -2*s1p*s1w0 + k2_j*s1w0 + selU_j + selV_j + k0_j

where (with y = logsim - s, s = 3.5 shift):
  X1[j, 0:192] = sum_l y[l, j] * [P | W0 | U][l, :]   (q-as-N accumulation)
  s1p/s1w0/selU = X1[j, g + t_j] for g = 0/64/128     (one-hot select)
  X2[c, j]     = sum_l y[l, j]^2 * V[l, c]            (q-major accumulation)
  sum_j selV_j = sum_{c,j} X2[c, j] * phT[c, j]       (single fused dot)
  K0 = sum_j k0_j is a pure-host constant.

U[l, c] = -0.2*h1[c]*P[l, c] + 0.2*(cnt[c]-1)*W1[c, l] and
V[l, c] = h0[c]*P[l, c] + (cnt[c]-1)*W0[c, l] absorb the linear terms whose
per-row coefficients depend on j only through t_j; only the bilinear
-2*s1p*s1w0 and the diag-dependent k2_j*s1w0 need explicit per-row selects.

Sharding: data-parallel over j across 8 cores (512 rows each). Every core
holds the full xT bf16 and computes its 512-wide j-slab of sim as
[l_tile(128) x j(512)] PSUM tiles. All matmul operands are bf16 (1 PE
cycle/row at any free size), so the x1 GEMM runs at N=192 unpadded and the
V GEMM shares its stationary operand across the whole l-tile. Per-j partial
losses are DMA'd out and summed on host (the "all-reduce" of a scalar).
"""

import numpy as np

N, D, KK, C = 4096, 128, 4, 64
NCORES = 8
JSH = N // NCORES          # 512 j rows per core
JCH = 4                    # j-chunks of 128 per core
LT = N // 128              # 32 l-tiles
QW = 192                   # x1 width: P(64) | W0(64) | U(64)
NSEL = 3                   # selections per row from X1
AUXW = 68                  # aux cols: 64 one-hot + 3 ktab + 1 pad
EPS = 1e-6
OMEGA = 0.1
KSC = float(np.float32(np.exp(-3.5)))        # Ln input scale (exactly f32)
SHIFT = float(-np.log(np.float64(KSC)))      # effective shift s = -ln(KSC)

_CACHE = {}


def _build_nc():
    import concourse.bass as bass
    import concourse.bacc as bacc
    import concourse.mybir as mybir
    import concourse.tile as tile
    from contextlib import ExitStack

    f32 = mybir.dt.float32
    bf16 = mybir.dt.bfloat16
    Ln = mybir.ActivationFunctionType.Ln
    mult = mybir.AluOpType.mult
    add = mybir.AluOpType.add
    AxX = mybir.AxisListType.X

    nc = bacc.Bacc("TRN2", target_bir_lowering=False, debug=False)
    xt = nc.dram_tensor("xt", [D, N], bf16, kind="ExternalInput")
    q = nc.dram_tensor("q", [LT, 128, QW], bf16, kind="ExternalInput")
    v = nc.dram_tensor("v", [LT, 128, C], bf16, kind="ExternalInput")
    # aux columns: [0:64] one-hot of t_j, [64:67] = [0, k2_j, 1]
    aux = nc.dram_tensor("aux", [JSH, AUXW], f32, kind="ExternalInput")
    pht = nc.dram_tensor("pht", [C, JSH], f32, kind="ExternalInput")
    lout = nc.dram_tensor("lout", [128, JCH + 1], f32, kind="ExternalOutput")

    with tile.TileContext(nc) as tc, ExitStack() as ctx:
        cpool = ctx.enter_context(tc.tile_pool(name="const", bufs=1))
        work = ctx.enter_context(tc.tile_pool(name="work", bufs=3))
        small = ctx.enter_context(tc.tile_pool(name="small", bufs=2))
        psim = ctx.enter_context(tc.tile_pool(name="psim", bufs=2, space="PSUM"))
        pacc = ctx.enter_context(tc.tile_pool(name="pacc", bufs=1, space="PSUM"))

        # ---- constants: xT, Q, V, aux resident in SBUF ----
        xt_sb = cpool.tile([D, N], bf16, tag="xt")
        for cchunk in range(4):
            sl = bass.ts(cchunk, 1024)
            nc.sync.dma_start(xt_sb[:, sl], xt[:, sl])
        q_sb = []
        v_sb = []
        for lt in range(LT):
            qt = cpool.tile([128, QW], bf16, tag=f"q{lt}", name=f"q_sb{lt}")
            nc.sync.dma_start(qt[:], q[lt])
            q_sb.append(qt)
            vt = cpool.tile([128, C], bf16, tag=f"v{lt}", name=f"v_sb{lt}")
            nc.sync.dma_start(vt[:], v[lt])
            v_sb.append(vt)
        aux_sb = []
        for jc in range(JCH):
            at = cpool.tile([128, AUXW], f32, tag=f"aux{jc}", name=f"aux{jc}")
            nc.sync.dma_start(at[:], aux[bass.ts(jc, 128), :])
            aux_sb.append(at)
        pht_sb = cpool.tile([C, JSH], f32, tag="pht")
        nc.sync.dma_start(pht_sb[:], pht[:])
        lbuf = cpool.tile([128, JCH + 1], f32, tag="lbuf")
        nc.gpsimd.memset(lbuf[:], 0.0)

        # ---- accumulators: one PSUM bank each ----
        acc = [
            pacc.tile([128, QW], f32, tag=f"acc{jc}", name=f"acc{jc}")
            for jc in range(JCH)
        ]
        xv = pacc.tile([C, JSH], f32, tag="xv")

        for lt in range(LT):
            simp = psim.tile([128, JSH], f32, tag="simp")
            nc.tensor.matmul(
                simp[:],
                xt_sb[:, bass.ts(lt, 128)],
                xt_sb[:, 0:JSH],
                start=True,
                stop=True,
            )
            ls = work.tile([128, JSH], bf16, tag="ls")
            nc.scalar.activation(ls[:], simp[:], Ln, scale=KSC)
            ls2 = work.tile([128, JSH], bf16, tag="ls2")
            nc.vector.tensor_mul(ls2[:], ls[:], ls[:])
            st = lt == 0
            sp = lt == LT - 1
            nc.tensor.matmul(xv[:], v_sb[lt][:], ls2[:], start=st, stop=sp)
            for jc in range(JCH):
                nc.tensor.matmul(
                    acc[jc][:], ls[:, bass.ts(jc, 128)], q_sb[lt][:],
                    start=st, stop=sp,
                )

        # ---- epilogue per 128-row j-chunk: 3 selections + combine ----
        for jc in range(JCH):
            pj3 = aux_sb[jc][:, 0:C].unsqueeze(1).broadcast_to([128, NSEL, C])
            kt3 = aux_sb[jc][:, C : C + NSEL]
            msel = small.tile([128, NSEL * C], f32, tag="msel", name=f"msel{jc}")
            nc.vector.tensor_mul(
                msel[:].rearrange("p (g c) -> p g c", g=NSEL),
                acc[jc][:].rearrange("p (g c) -> p g c", g=NSEL),
                pj3,
            )
            sels = small.tile([128, NSEL], f32, tag="sels", name=f"sels{jc}")
            nc.vector.reduce_sum(
                sels[:], msel[:].rearrange("p (g c) -> p g c", g=NSEL), axis=AxX
            )
            prod = small.tile([128, 1], f32, tag="prod", name=f"prod{jc}")
            nc.vector.scalar_tensor_tensor(
                out=prod[:], in0=sels[:, 0:1], scalar=-2.0, in1=sels[:, 1:2],
                op0=mult, op1=mult,
            )
            scr = small.tile([128, NSEL], f32, tag="scr", name=f"scr{jc}")
            nc.vector.tensor_mul(scr[:], sels[:], kt3)
            t3 = small.tile([128, 1], f32, tag="t3", name=f"t3{jc}")
            nc.vector.reduce_sum(t3[:], scr[:], axis=AxX)
            nc.vector.tensor_add(lbuf[:, jc : jc + 1], t3[:], prod[:])

        # ---- V-term: dot of X2 with phT, reduced along j ----
        mselv = small.tile([C, JSH], f32, tag="mselv")
        nc.vector.tensor_mul(mselv[:], xv[:], pht_sb[:])
        nc.vector.reduce_sum(lbuf[0:C, JCH : JCH + 1], mselv[:], axis=AxX)

        nc.sync.dma_start(lout[:], lbuf[:])
    nc.compile()
    return nc


def _host_prep(inputs, labels):
    import ml_dtypes

    bf16 = ml_dtypes.bfloat16
    x = np.ascontiguousarray(np.asarray(inputs, dtype=np.float32))
    lab = np.asarray(labels)
    t = lab[:, 0]

    m = np.arange(KK)
    lp = np.log(np.float64(OMEGA + EPS)) - np.log(
        np.float64(OMEGA) ** (KK - m + 1) + np.float64(EPS)
    )

    gr = np.arange(C)
    eq = lab[None, :, :] == gr[:, None, None]          # [C, N, KK]
    nm = np.stack(
        [
            ~eq[:, :, 3],
            eq[:, :, 3] & ~eq[:, :, 2],
            eq[:, :, 2] & ~eq[:, :, 1],
            eq[:, :, 1] & ~eq[:, :, 0],
        ]
    ).astype(np.float64)                                # [KK, C, N]
    w0 = nm.sum(0)                                      # [C, N]
    w1 = np.einsum("m,mcl->cl", lp, nm)
    w2 = np.einsum("m,mcl->cl", lp * lp, nm)
    ph = (t[:, None] == gr[None, :]).astype(np.float64)  # [N, C] one-hot t_l

    h0 = w0.sum(1)
    h1 = w1.sum(1)
    h2 = w2.sum(1)
    cnt = ph.sum(0)
    s = np.float64(SHIFT)

    # diag in logsim-domain from the bf16-rounded x the device will see
    xb = x.astype(bf16).astype(np.float64)
    diag = np.log((xb * xb).sum(1) + EPS)                # [N]

    a0 = h0[t]
    cc = cnt[t] - 1.0
    u1 = s * cnt[t] - diag
    u2 = s * s * cnt[t] - diag * diag
    u3 = s * h0[t] + 0.1 * h1[t]
    u4 = s * s * h0[t] + 0.2 * s * h1[t] + 0.01 * h2[t]
    k0 = (a0 * u2 - 2.0 * u1 * u3 + cc * u4).sum()       # host constant
    k2 = 2.0 * diag - 2.0 * s

    # fold t_j-only linear terms into U (for X1) and V (for X2)
    U = -0.2 * h1[None, :] * ph + 0.2 * (cnt[None, :] - 1.0) * w1.T
    V = h0[None, :] * ph + (cnt[None, :] - 1.0) * w0.T

    qm = np.concatenate([ph, w0.T, U], axis=1)           # [N, 192]

    auxf = np.zeros((N, AUXW), dtype=np.float32)
    auxf[:, 0:C] = ph
    auxf[:, C + 1] = k2
    auxf[:, C + 2] = 1.0

    xt = np.ascontiguousarray(x.T.astype(bf16))          # [D, N] bf16
    qmb = qm.astype(bf16)
    vb = V.astype(bf16)
    in_maps = []
    for cid in range(NCORES):
        sl = slice(cid * JSH, (cid + 1) * JSH)
        # rotate the l axis so this core's own j-shard sits at columns
        # 0:JSH — the kernel always matmuls against xt[:, 0:512]; the l
        # reduction (over all 4096) is rotation-invariant as long as q/v
        # rows rotate identically.
        xtc = np.ascontiguousarray(np.roll(xt, -cid * JSH, axis=1))
        qc = np.ascontiguousarray(
            np.roll(qmb, -cid * JSH, axis=0).reshape(LT, 128, QW)
        )
        vc = np.ascontiguousarray(
            np.roll(vb, -cid * JSH, axis=0).reshape(LT, 128, C)
        )
        in_maps.append(
            {
                "xt": xtc,
                "q": qc,
                "v": vc,
                "aux": np.ascontiguousarray(auxf[sl]),
                "pht": np.ascontiguousarray(
                    ph[sl].T.astype(np.float32)
                ),
            }
        )
    return in_maps, float(k0)


def _run(inputs, labels, trace=False, tmpdir=None):
    from concourse.bass_utils import run_bass_kernel_spmd

    if "nc" not in _CACHE:
        _CACHE["nc"] = _build_nc()
    in_maps, k0 = _host_prep(inputs, labels)
    res = run_bass_kernel_spmd(
        _CACHE["nc"], in_maps, core_ids=list(range(NCORES)),
        trace=trace, tmpdir=tmpdir,
    )
    loss = np.float64(k0)
    for r in res.results:
        lo = r["lout"].astype(np.float64)
        loss += lo[:, 0:JCH].sum() + lo[0:C, JCH].sum()
    return np.array(loss, dtype=np.float32), res


def kernel(inputs, labels):
    out, _ = _run(inputs, labels, trace=False)
    return out


# revision 8
# speedup vs baseline: 1.3154x; 1.0757x over previous
"""Trainium2 Bass kernel for the LogRatio loss (nn_LogRatio_14104672600201).

Math: the reference loss factorizes. Every masked reduction over
logsim[j, l] = log((X @ X.T)[j, l] + eps) has a mask that depends on j only
through targets[j] in [0, 64), so each row-reduction becomes a small GEMM
against matrices built on host from labels. After folding all shift / diag /
count terms into host-precomputed coefficients, the per-row loss is

  loss_j = -2*s1p*s1w0 + k2_j*s1w0 + selU_j + selV_j + k0_j

where (with y = logsim - s, s = 3.5 shift):
  X1[j, 0:192] = sum_l y[l, j] * [P | W0 | U][l, :]   (q-as-N accumulation)
  s1p/s1w0/selU = X1[j, g + t_j] for g = 0/64/128     (one-hot select)
  X2[c, j]     = sum_l y[l, j]^2 * V[l, c]            (q-major accumulation)
  sum_j selV_j = sum_{c,j} X2[c, j] * phT[c, j]       (single fused dot)
  K0 = sum_j k0_j is a pure-host constant.

U[l, c] = -0.2*h1[c]*P[l, c] + 0.2*(cnt[c]-1)*W1[c, l] and
V[l, c] = h0[c]*P[l, c] + (cnt[c]-1)*W0[c, l] absorb the linear terms whose
per-row coefficients depend on j only through t_j; only the bilinear
-2*s1p*s1w0 and the diag-dependent k2_j*s1w0 need explicit per-row selects.

Sharding: data-parallel over j across 8 cores (512 rows each). Every core
holds the full xT bf16 and computes its 512-wide j-slab of sim as
[l_tile(128) x j(512)] PSUM tiles. All matmul operands are bf16 (1 PE
cycle/row at any free size), so the x1 GEMM runs at N=192 unpadded and the
V GEMM shares its stationary operand across the whole l-tile. Per-j partial
losses are DMA'd out and summed on host (the "all-reduce" of a scalar).
"""

import numpy as np

N, D, KK, C = 4096, 128, 4, 64
NCORES = 8
JSH = N // NCORES          # 512 j rows per core
JCH = 4                    # j-chunks of 128 per core
LT = N // 128              # 32 l-tiles
QW = 192                   # x1 width: P(64) | W0(64) | U(64)
NSEL = 3                   # selections per row from X1
AUXW = 68                  # aux cols: 64 one-hot + 3 ktab + 1 pad
EPS = 1e-6
OMEGA = 0.1
KSC = float(np.float32(np.exp(-3.5)))        # Ln input scale (exactly f32)
SHIFT = float(-np.log(np.float64(KSC)))      # effective shift s = -ln(KSC)

_CACHE = {}


def _build_nc():
    import concourse.bass as bass
    import concourse.bacc as bacc
    import concourse.mybir as mybir
    import concourse.tile as tile
    from contextlib import ExitStack

    f32 = mybir.dt.float32
    bf16 = mybir.dt.bfloat16
    Ln = mybir.ActivationFunctionType.Ln
    mult = mybir.AluOpType.mult
    add = mybir.AluOpType.add
    AxX = mybir.AxisListType.X

    QV = QW + C                # 256: packed [q(192) | v(64)] per l-tile

    nc = bacc.Bacc("TRN2", target_bir_lowering=False, debug=False)
    xt = nc.dram_tensor("xt", [D, N], bf16, kind="ExternalInput")
    # qv is a pre-packed SBUF image: row p holds [q_lt(192) | v_lt(64)] for
    # l = lt*128 + p, all lt — so one contiguous DMA per chunk.
    qv = nc.dram_tensor("qv", [128, LT * QV], bf16, kind="ExternalInput")
    # aux: per-partition p, JCH blocks of [one-hot(64) | 0, k2, 1 | pad]
    aux = nc.dram_tensor("aux", [128, JCH * AUXW], f32, kind="ExternalInput")
    pht = nc.dram_tensor("pht", [C, JSH], f32, kind="ExternalInput")
    lout = nc.dram_tensor("lout", [128, JCH + 1], f32, kind="ExternalOutput")

    with tile.TileContext(nc) as tc, ExitStack() as ctx:
        cpool = ctx.enter_context(tc.tile_pool(name="const", bufs=1))
        work = ctx.enter_context(tc.tile_pool(name="work", bufs=4))
        small = ctx.enter_context(tc.tile_pool(name="small", bufs=2))
        psim = ctx.enter_context(tc.tile_pool(name="psim", bufs=2, space="PSUM"))
        pacc = ctx.enter_context(tc.tile_pool(name="pacc", bufs=1, space="PSUM"))

        # ---- constants: xT, QV, aux resident in SBUF ----
        xt_sb = cpool.tile([D, N], bf16, tag="xt")
        for cchunk in range(4):
            sl = bass.ts(cchunk, 1024)
            nc.sync.dma_start(xt_sb[:, sl], xt[:, sl])
        qv_sb = cpool.tile([128, LT * QV], bf16, tag="qv")
        for cchunk in range(4):
            sl = bass.ts(cchunk, LT * QV // 4)
            nc.sync.dma_start(qv_sb[:, sl], qv[:, sl])
        q_sb = [qv_sb[:, lt * QV : lt * QV + QW] for lt in range(LT)]
        v_sb = [qv_sb[:, lt * QV + QW : (lt + 1) * QV] for lt in range(LT)]
        aux_all = cpool.tile([128, JCH * AUXW], f32, tag="aux")
        nc.sync.dma_start(aux_all[:], aux[:])
        aux_sb = [aux_all[:, jc * AUXW : (jc + 1) * AUXW] for jc in range(JCH)]
        pht_sb = cpool.tile([C, JSH], f32, tag="pht")
        nc.sync.dma_start(pht_sb[:], pht[:])
        lbuf = cpool.tile([128, JCH + 1], f32, tag="lbuf")
        nc.gpsimd.memset(lbuf[:], 0.0)

        # ---- accumulators: one PSUM bank each ----
        acc = [
            pacc.tile([128, QW], f32, tag=f"acc{jc}", name=f"acc{jc}")
            for jc in range(JCH)
        ]
        xv = pacc.tile([C, JSH], f32, tag="xv")

        for lt in range(LT):
            simp = psim.tile([128, JSH], f32, tag="simp")
            nc.tensor.matmul(
                simp[:],
                xt_sb[:, bass.ts(lt, 128)],
                xt_sb[:, 0:JSH],
                start=True,
                stop=True,
            )
            ls = work.tile([128, JSH], bf16, tag="ls")
            nc.scalar.activation(ls[:], simp[:], Ln, scale=KSC)
            ls2 = work.tile([128, JSH], bf16, tag="ls2")
            nc.vector.tensor_mul(ls2[:], ls[:], ls[:])
            st = lt == 0
            sp = lt == LT - 1
            nc.tensor.matmul(xv[:], v_sb[lt], ls2[:], start=st, stop=sp)
            for jc in range(JCH):
                nc.tensor.matmul(
                    acc[jc][:], ls[:, bass.ts(jc, 128)], q_sb[lt],
                    start=st, stop=sp,
                )

        # ---- epilogue per 128-row j-chunk: 3 selections + combine ----
        for jc in range(JCH):
            pj3 = aux_sb[jc][:, 0:C].unsqueeze(1).broadcast_to([128, NSEL, C])
            kt3 = aux_sb[jc][:, C : C + NSEL]
            msel = small.tile([128, NSEL * C], f32, tag="msel", name=f"msel{jc}")
            nc.vector.tensor_mul(
                msel[:].rearrange("p (g c) -> p g c", g=NSEL),
                acc[jc][:].rearrange("p (g c) -> p g c", g=NSEL),
                pj3,
            )
            sels = small.tile([128, NSEL], f32, tag="sels", name=f"sels{jc}")
            nc.vector.reduce_sum(
                sels[:], msel[:].rearrange("p (g c) -> p g c", g=NSEL), axis=AxX
            )
            prod = small.tile([128, 1], f32, tag="prod", name=f"prod{jc}")
            nc.vector.scalar_tensor_tensor(
                out=prod[:], in0=sels[:, 0:1], scalar=-2.0, in1=sels[:, 1:2],
                op0=mult, op1=mult,
            )
            scr = small.tile([128, NSEL], f32, tag="scr", name=f"scr{jc}")
            nc.vector.tensor_mul(scr[:], sels[:], kt3)
            t3 = small.tile([128, 1], f32, tag="t3", name=f"t3{jc}")
            nc.vector.reduce_sum(t3[:], scr[:], axis=AxX)
            nc.vector.tensor_add(lbuf[:, jc : jc + 1], t3[:], prod[:])

        # ---- V-term: dot of X2 with phT, reduced along j ----
        mselv = small.tile([C, JSH], f32, tag="mselv")
        nc.vector.tensor_mul(mselv[:], xv[:], pht_sb[:])
        nc.vector.reduce_sum(lbuf[0:C, JCH : JCH + 1], mselv[:], axis=AxX)

        nc.sync.dma_start(lout[:], lbuf[:])
    nc.compile()
    return nc


def _host_prep(inputs, labels):
    import ml_dtypes

    bf16 = ml_dtypes.bfloat16
    x = np.ascontiguousarray(np.asarray(inputs, dtype=np.float32))
    lab = np.asarray(labels)
    t = lab[:, 0]

    m = np.arange(KK)
    lp = np.log(np.float64(OMEGA + EPS)) - np.log(
        np.float64(OMEGA) ** (KK - m + 1) + np.float64(EPS)
    )

    gr = np.arange(C)
    eq = lab[None, :, :] == gr[:, None, None]          # [C, N, KK]
    nm = np.stack(
        [
            ~eq[:, :, 3],
            eq[:, :, 3] & ~eq[:, :, 2],
            eq[:, :, 2] & ~eq[:, :, 1],
            eq[:, :, 1] & ~eq[:, :, 0],
        ]
    ).astype(np.float64)                                # [KK, C, N]
    w0 = nm.sum(0)                                      # [C, N]
    w1 = np.einsum("m,mcl->cl", lp, nm)
    w2 = np.einsum("m,mcl->cl", lp * lp, nm)
    ph = (t[:, None] == gr[None, :]).astype(np.float64)  # [N, C] one-hot t_l

    h0 = w0.sum(1)
    h1 = w1.sum(1)
    h2 = w2.sum(1)
    cnt = ph.sum(0)
    s = np.float64(SHIFT)

    # diag in logsim-domain from the bf16-rounded x the device will see
    xb = x.astype(bf16).astype(np.float64)
    diag = np.log((xb * xb).sum(1) + EPS)                # [N]

    a0 = h0[t]
    cc = cnt[t] - 1.0
    u1 = s * cnt[t] - diag
    u2 = s * s * cnt[t] - diag * diag
    u3 = s * h0[t] + 0.1 * h1[t]
    u4 = s * s * h0[t] + 0.2 * s * h1[t] + 0.01 * h2[t]
    k0 = (a0 * u2 - 2.0 * u1 * u3 + cc * u4).sum()       # host constant
    k2 = 2.0 * diag - 2.0 * s

    # fold t_j-only linear terms into U (for X1) and V (for X2)
    U = -0.2 * h1[None, :] * ph + 0.2 * (cnt[None, :] - 1.0) * w1.T
    V = h0[None, :] * ph + (cnt[None, :] - 1.0) * w0.T

    # packed [q(192) | v(64)] per l-row, bf16
    qvm = np.concatenate([ph, w0.T, U, V], axis=1).astype(bf16)  # [N, 256]

    auxf = np.zeros((N, AUXW), dtype=np.float32)
    auxf[:, 0:C] = ph
    auxf[:, C + 1] = k2
    auxf[:, C + 2] = 1.0
    # SBUF image: partition p holds the 4 j-chunks' blocks side by side
    auxp = np.ascontiguousarray(
        auxf.reshape(NCORES, JCH, 128, AUXW).transpose(0, 2, 1, 3)
        .reshape(NCORES, 128, JCH * AUXW)
    )

    xt = np.ascontiguousarray(x.T.astype(bf16))          # [D, N] bf16
    in_maps = []
    for cid in range(NCORES):
        sl = slice(cid * JSH, (cid + 1) * JSH)
        # rotate the l axis so this core's own j-shard sits at columns
        # 0:JSH — the kernel always matmuls against xt[:, 0:512]; the l
        # reduction (over all 4096) is rotation-invariant as long as qv
        # rows rotate identically.
        xtc = np.ascontiguousarray(np.roll(xt, -cid * JSH, axis=1))
        # SBUF image: partition p holds [qv block of l-tile lt] for all lt
        qvc = np.ascontiguousarray(
            np.roll(qvm, -cid * JSH, axis=0)
            .reshape(LT, 128, QW + C).transpose(1, 0, 2).reshape(128, -1)
        )
        in_maps.append(
            {
                "xt": xtc,
                "qv": qvc,
                "aux": auxp[cid],
                "pht": np.ascontiguousarray(
                    ph[sl].T.astype(np.float32)
                ),
            }
        )
    return in_maps, float(k0)


def _run(inputs, labels, trace=False, tmpdir=None):
    from concourse.bass_utils import run_bass_kernel_spmd

    if "nc" not in _CACHE:
        _CACHE["nc"] = _build_nc()
    in_maps, k0 = _host_prep(inputs, labels)
    res = run_bass_kernel_spmd(
        _CACHE["nc"], in_maps, core_ids=list(range(NCORES)),
        trace=trace, tmpdir=tmpdir,
    )
    loss = np.float64(k0)
    for r in res.results:
        lo = r["lout"].astype(np.float64)
        loss += lo[:, 0:JCH].sum() + lo[0:C, JCH].sum()
    return np.array(loss, dtype=np.float32), res


def kernel(inputs, labels):
    out, _ = _run(inputs, labels, trace=False)
    return out
